# revision 7
# baseline (speedup 1.0000x reference)
# Multi-head causal attention (B=4, T=2048, D=1024, H=16, dk=64), fp32 in/out.
#
# Sharding: 8 cores = 4 batches x 2 head-groups (8 heads / 512 cols each).
# Each core computes a partial output  y_g @ wo_g  for its batch; the host
# sums the two head-group partials per batch and adds the constant row
# (bv @ wo + bo), which is exact because softmax rows sum to 1.
#
# v2 design (all-bf16 datapath, fp32 accumulation in PSUM):
#  - x is pre-converted to bf16 on the host and DMA-transposed (xbar) into
#    SBUF as xT [d, i] chunks - no PE transposes, no DVE copy-back.
#  - scores: row-packed head pairs (K=64 at PE rows 0/64) -> one PSUM pair
#    tile [128, 2x512]; mask add batched; one Exp activation per pair-chunk.
#  - av: column-packed head pairs (M=64 at PE cols 0/64) -> psy [128, 512]
#    holds both heads, so no cross-partition shuffle of odd heads.
#  - softmax denominators: M=32 sel-matmuls accumulated into a single PSUM
#    bank (col-tiled positions 0/32/64/96 x 2 rows), no DVE row gathering.
#  - per-ib interleaved emission (A1 ahead of B0 etc) so projection matmuls
#    fill TensorE while attention is ScalarE(exp)-bound.

import numpy as np

B, T, D, H, DK = 4, 2048, 1024, 16, 64
NCORES = 8
G = 2               # head groups (tensor-parallel over heads)
C = D // G          # 512 columns per core = 8 heads
NH = C // DK        # heads per core = 8
NIB = T // 512      # 4 query blocks of 512
NJC = T // 128      # 16 key chunks of 128
SCALE = 1.0 / 8.0   # 1/sqrt(dk)

MM_MODE = "bf16"    # kept for test.py compatibility


def _den_row(h):
    # head h's denominator lives at PSUM partition 32*(h%4) + h//4
    return 32 * (h % 4) + h // 4


def build_nc(mm_mode=MM_MODE, n_reps=1):
    from contextlib import ExitStack

    import concourse.bass as bass
    import concourse.mybir as mybir
    import concourse.tile as tile
    from concourse import bacc

    f32 = mybir.dt.float32
    bf16 = mybir.dt.bfloat16
    AF = mybir.ActivationFunctionType

    nc = bacc.Bacc("TRN2", target_bir_lowering=False, debug=False,
                   num_devices=NCORES)

    x_d = nc.dram_tensor("x", [T, D], bf16, kind="ExternalInput").ap()
    wq_d = nc.dram_tensor("wq", [D, C], bf16, kind="ExternalInput").ap()
    wk_d = nc.dram_tensor("wk", [D, C], bf16, kind="ExternalInput").ap()
    wv_d = nc.dram_tensor("wv", [D, C], bf16, kind="ExternalInput").ap()
    wo_d = nc.dram_tensor("wo", [C, D], bf16, kind="ExternalInput").ap()
    bq_d = nc.dram_tensor("bq", [C, 1], f32, kind="ExternalInput").ap()
    bk_d = nc.dram_tensor("bk", [C, 1], f32, kind="ExternalInput").ap()
    msk_d = nc.dram_tensor("invmask2", [128, 256], f32, kind="ExternalInput").ap()
    selp_d = nc.dram_tensor("selp", [4, 128, 128], bf16, kind="ExternalInput").ap()
    ones2_d = nc.dram_tensor("sel32", [2, 128, 32], bf16, kind="ExternalInput").ap()
    out_d = nc.dram_tensor("out", [T, D], f32, kind="ExternalOutput").ap()

    with tile.TileContext(nc) as tc, ExitStack() as st:
        pers = st.enter_context(tc.tile_pool(name="pers", bufs=1))
        sb = st.enter_context(tc.tile_pool(name="sb", bufs=1))
        ps = st.enter_context(tc.tile_pool(name="ps", bufs=1, space="PSUM"))

        # persistent constants
        kT = [pers.tile([128, T], bf16, name=f"kT{cc}", tag=f"kT{cc}")
              for cc in range(4)]
        v_ext = pers.tile([128, NJC, NH, DK], bf16, name="v_ext", tag="v_ext")
        bq_sb = pers.tile([128, 4], f32, name="bq_sb", tag="bq_sb")
        bk_sb = pers.tile([128, 4], f32, name="bk_sb", tag="bk_sb")
        invm = pers.tile([128, 2, 128], f32, name="invm", tag="invm")
        selp = pers.tile([128, 4, 128], bf16, name="selp", tag="selp")
        sel32 = pers.tile([128, 2, 32], bf16, name="sel32", tag="sel32")
        rec = pers.tile([128, 512], bf16, name="rec", tag="rec")

        nc.sync.dma_start(invm[:], msk_d.rearrange("p (t q) -> p t q", t=2))
        for hp in range(4):
            nc.sync.dma_start(selp[:, hp, :], selp_d[hp])
        for i in range(2):
            nc.sync.dma_start(sel32[:, i, :], ones2_d[i])
        for cc in range(4):
            nc.sync.dma_start(bq_sb[:, cc:cc + 1], bq_d[cc * 128:(cc + 1) * 128, :])
            nc.sync.dma_start(bk_sb[:, cc:cc + 1], bk_d[cc * 128:(cc + 1) * 128, :])
        nc.vector.memset(rec[:], 0.0)

        for rep_ in range(n_reps):
            r = f"r{rep_}"
            # weights (re-loaded per rep; rings give WAR deps automatically)
            wq_sb = [sb.tile([128, C], bf16, name=f"{r}_wq{dc}", tag=f"wq{dc}")
                     for dc in range(8)]
            wk_sb = [sb.tile([128, C], bf16, name=f"{r}_wk{dc}", tag=f"wk{dc}")
                     for dc in range(8)]
            wv_sb = [sb.tile([128, C], bf16, name=f"{r}_wv{dc}", tag=f"wv{dc}")
                     for dc in range(8)]
            wo_sb = [sb.tile([128, D], bf16, name=f"{r}_wo{cc}", tag=f"wo{cc}")
                     for cc in range(4)]
            for dc in range(8):
                nc.sync.dma_start(wq_sb[dc][:], wq_d[dc * 128:(dc + 1) * 128, :])
                nc.sync.dma_start(wk_sb[dc][:], wk_d[dc * 128:(dc + 1) * 128, :])
                nc.sync.dma_start(wv_sb[dc][:], wv_d[dc * 128:(dc + 1) * 128, :])
            for cc in range(4):
                nc.sync.dma_start(wo_sb[cc][:], wo_d[cc * 128:(cc + 1) * 128, :])

            qT = {}   # qT[(ib, cc)] live for one ib (ring bufs=2)

            def stage_a(ib):
                # xT via DMA xbar transpose: x[ib block, dc cols] -> [128, 512]
                xT = [sb.tile([128, 512], bf16, name=f"{r}_xT_{ib}_{dc}",
                              tag=f"xT{dc}", bufs=2) for dc in range(8)]
                for dc in range(8):
                    nc.sync.dma_start(
                        xT[dc][:],
                        x_d[ib * 512:(ib + 1) * 512, dc * 128:(dc + 1) * 128],
                        transpose=True)
                # q/k projections -> qT (transposed, bf16, +bias), kT
                for cc in range(4):
                    pq = ps.tile([128, 512], f32, name=f"{r}_pq_{ib}_{cc}",
                                 tag="proj", bufs=2)
                    for dc in range(8):
                        nc.tensor.matmul(pq[:], wq_sb[dc][:, cc * 128:(cc + 1) * 128],
                                         xT[dc][:], start=(dc == 0), stop=(dc == 7))
                    qt = sb.tile([128, 512], bf16, name=f"{r}_qT_{ib}_{cc}",
                                 tag=f"qT{cc}", bufs=2)
                    nc.vector.tensor_scalar_add(qt[:], pq[:], bq_sb[:, cc:cc + 1])
                    qT[(ib, cc)] = qt
                    pk = ps.tile([128, 512], f32, name=f"{r}_pk_{ib}_{cc}",
                                 tag="proj", bufs=2)
                    for dc in range(8):
                        nc.tensor.matmul(pk[:], wk_sb[dc][:, cc * 128:(cc + 1) * 128],
                                         xT[dc][:], start=(dc == 0), stop=(dc == 7))
                    nc.vector.tensor_scalar_add(
                        kT[cc][:, ib * 512:(ib + 1) * 512], pk[:],
                        bk_sb[:, cc:cc + 1])
                # v projection (natural layout; no bias - folded into host const)
                for isub in range(4):
                    pv = ps.tile([128, 512], f32, name=f"{r}_pv_{ib}_{isub}",
                                 tag="proj", bufs=2)
                    for dc in range(8):
                        nc.tensor.matmul(pv[:], xT[dc][:, isub * 128:(isub + 1) * 128],
                                         wv_sb[dc][:], start=(dc == 0), stop=(dc == 7))
                    nc.vector.tensor_copy(
                        v_ext[:, ib * 4 + isub, :, :],
                        pv[:].rearrange("p (h d) -> p h d", d=DK))

            def stage_b(ib):
                njc = 4 * (ib + 1)
                den = ps.tile([128, 512], f32, name=f"{r}_den_{ib}", tag="den",
                              bufs=1)
                yt = []
                for hp in range(4):
                    h0, h1 = 2 * hp, 2 * hp + 1
                    psy = ps.tile([128, 512], f32, name=f"{r}_psy_{ib}_{hp}",
                                  tag="psy", bufs=1)
                    for jc in range(njc):
                        o = max(0, jc - 4 * ib)
                        i0 = o * 128
                        nw = 512 - i0
                        pss = ps.tile([128, 1024], f32, name=f"{r}_pss_{ib}_{hp}_{jc}",
                                      tag="pss", bufs=2)
                        nc.tensor.matmul(
                            pss[:, 0:nw],
                            kT[hp][0:64, jc * 128:(jc + 1) * 128],
                            qT[(ib, hp)][0:64, i0:512],
                            start=True, stop=True, tile_position=(0, 0))
                        nc.tensor.matmul(
                            pss[:, 512:512 + nw],
                            kT[hp][64:128, jc * 128:(jc + 1) * 128],
                            qT[(ib, hp)][64:128, i0:512],
                            start=True, stop=True, tile_position=(64, 0))
                        pv2 = pss[:].rearrange("p (t q) -> p t q", t=2)
                        if jc >= 4 * ib:
                            nc.vector.tensor_add(pv2[:, :, 0:128],
                                                 pv2[:, :, 0:128], invm[:])
                        et = sb.tile([128, 1024], bf16, name=f"{r}_et_{ib}_{hp}_{jc}",
                                     tag="et", bufs=4)
                        et2 = et[:].rearrange("p (t q) -> p t q", t=2)
                        nc.scalar.activation(et2[:, :, 0:nw], pv2[:, :, 0:nw],
                                             AF.Exp, scale=SCALE)
                        # each den region (32 partitions) must see its own
                        # start=True: pairs 0/1 clear the four regions, pairs
                        # 2/3 accumulate (their sel32 col for the partner head
                        # is zero, so nothing real is overwritten)
                        first = (hp < 2 and jc == 0)
                        last = (hp >= 2 and jc == njc - 1)
                        nc.tensor.matmul(
                            psy[0:64, i0:512], v_ext[:, jc, h0, :], et[:, 0:nw],
                            start=(jc == 0), stop=(jc == njc - 1),
                            skip_group_check=True, tile_position=(0, 0))
                        nc.tensor.matmul(
                            psy[64:128, i0:512], v_ext[:, jc, h1, :],
                            et[:, 512:512 + nw],
                            start=(jc == 0), stop=(jc == njc - 1),
                            skip_group_check=True, tile_position=(0, 64))
                        r0 = 32 * (h0 % 4)
                        r1 = 32 * (h1 % 4)
                        nc.tensor.matmul(
                            den[r0:r0 + 32, i0:512], sel32[:, h0 // 4, :],
                            et[:, 0:nw],
                            start=first, stop=last, skip_group_check=True,
                            tile_position=(0, r0))
                        nc.tensor.matmul(
                            den[r1:r1 + 32, i0:512], sel32[:, h1 // 4, :],
                            et[:, 512:512 + nw],
                            start=first, stop=last, skip_group_check=True,
                            tile_position=(0, r1))
                    # stash y0 pair to SBUF (frees psy for next pair)
                    y = sb.tile([128, 512], bf16, name=f"{r}_yt_{ib}_{hp}",
                                tag=f"yt{hp}", bufs=2)
                    nc.vector.tensor_copy(y[:], psy[:])
                    yt.append(y)
                # reciprocal of the whole den bank (every row finite: the
                # sel32 duplicate-ones columns fill unused rows with sums)
                with nc.allow_low_precision(reason="1/den in bf16 is plenty"):
                    nc.vector.reciprocal(rec[:], den[:])
                packed = []
                for hp in range(4):
                    pb = ps.tile([128, 512], f32, name=f"{r}_pb_{ib}_{hp}",
                                 tag="proj", bufs=2)
                    nc.tensor.matmul(pb[:], selp[:, hp, :], rec[:],
                                     start=True, stop=True)
                    pk2 = sb.tile([128, 512], bf16, name=f"{r}_pk2_{ib}_{hp}",
                                  tag=f"packed{hp}", bufs=2)
                    nc.vector.tensor_mul(pk2[:], yt[hp][:], pb[:])
                    packed.append(pk2)
                # output projection
                for isub in range(4):
                    row0 = (ib * 4 + isub) * 128
                    osb = sb.tile([128, 1024], f32, name=f"{r}_osb_{ib}_{isub}",
                                  tag="osb", bufs=2)
                    for nb in range(2):
                        pso = ps.tile([128, 512], f32, name=f"{r}_pso_{ib}_{isub}_{nb}",
                                      tag="proj", bufs=2)
                        for cc in range(4):
                            nc.tensor.matmul(
                                pso[:],
                                packed[cc][:, isub * 128:(isub + 1) * 128],
                                wo_sb[cc][:, nb * 512:(nb + 1) * 512],
                                start=(cc == 0), stop=(cc == 3))
                        if nb == 0:
                            nc.scalar.copy(osb[:, 0:512], pso[:])
                        else:
                            nc.vector.tensor_copy(osb[:, 512:1024], pso[:])
                    nc.sync.dma_start(out_d[row0:row0 + 128, :], osb[:])

            # emission order: A0 A1 B0 A2 B1 A3 B2 B3 (A runs one ib ahead
            # so projection matmuls overlap exp-bound attention phases)
            stage_a(0)
            stage_a(1)
            stage_b(0)
            stage_a(2)
            stage_b(1)
            stage_a(3)
            stage_b(2)
            stage_b(3)

    nc.compile()
    return nc


def make_in_maps(x, wq, bq, wk, bk, wv, bv, wo, bo):
    import ml_dtypes
    bf16 = ml_dtypes.bfloat16

    jj = np.arange(128)[:, None]
    ii = np.arange(128)[None, :]
    inv_mask = np.where(jj > ii, -1e9, 0.0).astype(np.float32)
    invm2 = np.concatenate([inv_mask, inv_mask], axis=1)  # [128, 256]

    selp = np.zeros((4, 128, 128), dtype=np.float32)
    for hp in range(4):
        h0, h1 = 2 * hp, 2 * hp + 1
        selp[hp, _den_row(h0), 0:64] = 1.0
        selp[hp, _den_row(h1), 64:128] = 1.0
    # sel32[v]: column j routes et-colsums into den partition row 32g+j.
    # col v is head v's real denominator slot; col (1-v) is the other head's
    # (zeroed); cols 2..31 get harmless sums so 1/den stays finite bank-wide.
    sel32 = np.ones((2, 128, 32), dtype=np.float32)
    sel32[0, :, 1] = 0.0
    sel32[1, :, 0] = 0.0

    x = np.asarray(x, np.float32)
    in_maps = []
    for c in range(NCORES):
        b, g = c // G, c % G
        cs = slice(g * C, (g + 1) * C)
        in_maps.append({
            "x": np.ascontiguousarray(x[b]).astype(bf16),
            "wq": np.ascontiguousarray(wq[:, cs]).astype(bf16),
            "wk": np.ascontiguousarray(wk[:, cs]).astype(bf16),
            "wv": np.ascontiguousarray(wv[:, cs]).astype(bf16),
            "wo": np.ascontiguousarray(wo[cs, :]).astype(bf16),
            "bq": np.ascontiguousarray(bq[cs].reshape(C, 1)).astype(np.float32),
            "bk": np.ascontiguousarray(bk[cs].reshape(C, 1)).astype(np.float32),
            "invmask2": invm2,
            "selp": selp.astype(bf16),
            "sel32": sel32.astype(bf16),
        })
    return in_maps


_NC_CACHE = {}


def _get_nc(mm_mode=MM_MODE):
    if mm_mode not in _NC_CACHE:
        _NC_CACHE[mm_mode] = build_nc(mm_mode)
    return _NC_CACHE[mm_mode]


def kernel(x, mask, wq, bq, wk, bk, wv, bv, wo, bo, _trace=False, _results=None):
    from concourse.bass_utils import run_bass_kernel_spmd

    nc = _get_nc()
    in_maps = make_in_maps(np.asarray(x), np.asarray(wq), np.asarray(bq),
                           np.asarray(wk), np.asarray(bk), np.asarray(wv),
                           np.asarray(bv), np.asarray(wo), np.asarray(bo))
    res = run_bass_kernel_spmd(nc, in_maps, core_ids=list(range(NCORES)),
                               trace=_trace)
    if _results is not None:
        _results.append(res)
    # constant row: y += bv (since attn rows sum to 1)  =>  out += bv@wo + bo
    row_const = (np.asarray(bv, np.float64) @ np.asarray(wo, np.float64)
                 + np.asarray(bo, np.float64)).astype(np.float32)
    out = np.empty((B, T, D), dtype=np.float32)
    for b in range(B):
        out[b] = (res.results[2 * b]["out"] + res.results[2 * b + 1]["out"]
                  + row_const)
    return out


# revision 10
# speedup vs baseline: 1.9120x; 1.9120x over previous
# Multi-head causal attention (B=4, T=2048, D=1024, H=16, dk=64), fp32 in/out.
#
# Sharding: 8 cores = 4 batches x 2 head-groups (8 heads / 512 cols each).
# Each core computes a partial output  y_g @ wo_g  for its batch; the host
# sums the two head-group partials per batch and adds the constant row
# (bv @ wo + bo), which is exact because softmax rows sum to 1.
#
# v2 design (all-bf16 datapath, fp32 accumulation in PSUM):
#  - x is pre-converted to bf16 on the host and DMA-transposed (xbar) into
#    SBUF as xT [d, i] chunks - no PE transposes, no DVE copy-back.
#  - scores: row-packed head pairs (K=64 at PE rows 0/64) -> one PSUM pair
#    tile [128, 2x512]; mask add batched; one Exp activation per pair-chunk.
#  - av: column-packed head pairs (M=64 at PE cols 0/64) -> psy [128, 512]
#    holds both heads, so no cross-partition shuffle of odd heads.
#  - softmax denominators: M=32 sel-matmuls accumulated into a single PSUM
#    bank (col-tiled positions 0/32/64/96 x 2 rows), no DVE row gathering.
#  - per-ib interleaved emission (A1 ahead of B0 etc) so projection matmuls
#    fill TensorE while attention is ScalarE(exp)-bound.

import numpy as np

B, T, D, H, DK = 4, 2048, 1024, 16, 64
NCORES = 8
G = 2               # head groups (tensor-parallel over heads)
C = D // G          # 512 columns per core = 8 heads
NH = C // DK        # heads per core = 8
NIB = T // 512      # 4 query blocks of 512
NJC = T // 128      # 16 key chunks of 128
SCALE = 1.0 / 8.0   # 1/sqrt(dk)

MM_MODE = "bf16"    # kept for test.py compatibility


def _den_row(h):
    # head h's denominator lives at PSUM partition 32*(h%4) + h//4
    return 32 * (h % 4) + h // 4


def build_nc(mm_mode=MM_MODE, n_reps=1):
    from contextlib import ExitStack

    import concourse.bass as bass
    import concourse.mybir as mybir
    import concourse.tile as tile
    from concourse import bacc

    f32 = mybir.dt.float32
    bf16 = mybir.dt.bfloat16
    AF = mybir.ActivationFunctionType

    nc = bacc.Bacc("TRN2", target_bir_lowering=False, debug=False,
                   num_devices=NCORES)

    x_d = nc.dram_tensor("x", [T, D], bf16, kind="ExternalInput").ap()
    wq_d = nc.dram_tensor("wq", [D, C], bf16, kind="ExternalInput").ap()
    wk_d = nc.dram_tensor("wk", [D, C], bf16, kind="ExternalInput").ap()
    wv_d = nc.dram_tensor("wv", [D, C], bf16, kind="ExternalInput").ap()
    wo_d = nc.dram_tensor("wo", [C, D], bf16, kind="ExternalInput").ap()
    bq_d = nc.dram_tensor("bq", [C, 1], f32, kind="ExternalInput").ap()
    bk_d = nc.dram_tensor("bk", [C, 1], f32, kind="ExternalInput").ap()
    msk_d = nc.dram_tensor("invmask2", [128, 256], f32, kind="ExternalInput").ap()
    selp_d = nc.dram_tensor("selp", [4, 128, 128], bf16, kind="ExternalInput").ap()
    ones2_d = nc.dram_tensor("sel32", [2, 128, 32], bf16, kind="ExternalInput").ap()
    out_d = nc.dram_tensor("out", [T, D], f32, kind="ExternalOutput").ap()

    with tile.TileContext(nc) as tc, ExitStack() as st:
        pers = st.enter_context(tc.tile_pool(name="pers", bufs=1))
        sb = st.enter_context(tc.tile_pool(name="sb", bufs=1))
        ps = st.enter_context(tc.tile_pool(name="ps", bufs=1, space="PSUM"))

        # persistent constants (each loaded with a single batched DMA)
        bq_sb = pers.tile([128, 4], f32, name="bq_sb", tag="bq_sb")
        bk_sb = pers.tile([128, 4], f32, name="bk_sb", tag="bk_sb")
        invm = pers.tile([128, 2, 128], f32, name="invm", tag="invm")
        selp = pers.tile([128, 4, 128], bf16, name="selp", tag="selp")
        sel32 = pers.tile([128, 2, 32], bf16, name="sel32", tag="sel32")

        nc.sync.dma_start(invm[:], msk_d.rearrange("p (t q) -> p t q", t=2))
        nc.sync.dma_start(selp[:], selp_d.rearrange("hp p m -> p hp m"))
        nc.sync.dma_start(sel32[:], ones2_d.rearrange("i p m -> p i m"))
        nc.sync.dma_start(bq_sb[:], bq_d.rearrange("(cc p) one -> p (cc one)", p=128))
        nc.sync.dma_start(bk_sb[:], bk_d.rearrange("(cc p) one -> p (cc one)", p=128))

        for rep_ in range(n_reps):
            r = f"r{rep_}"
            # first block of x transposes goes out before anything else so
            # the PE can start projecting ASAP
            xT0 = [sb.tile([128, 512], bf16, name=f"{r}_xT_0_{dc}",
                           tag=f"xT{dc}", bufs=2) for dc in range(8)]
            for dc in range(8):
                nc.sync.dma_start(
                    xT0[dc][:], x_d[0:512, dc * 128:(dc + 1) * 128],
                    transpose=True)
            # weights: one batched DMA per tensor, double-buffered so the
            # next rep's loads overlap this rep's tail
            wq_sb = sb.tile([128, 8, C], bf16, name=f"{r}_wq", tag="wq", bufs=2)
            wk_sb = sb.tile([128, 8, C], bf16, name=f"{r}_wk", tag="wk", bufs=2)
            wv_sb = sb.tile([128, 8, C], bf16, name=f"{r}_wv", tag="wv", bufs=2)
            wo_sb = sb.tile([128, 4, D], bf16, name=f"{r}_wo", tag="wo", bufs=2)
            nc.sync.dma_start(wq_sb[:], wq_d.rearrange("(dc p) n -> p dc n", p=128))
            nc.sync.dma_start(wk_sb[:], wk_d.rearrange("(dc p) n -> p dc n", p=128))
            nc.sync.dma_start(wv_sb[:], wv_d.rearrange("(dc p) n -> p dc n", p=128))
            nc.sync.dma_start(wo_sb[:], wo_d.rearrange("(cc p) n -> p cc n", p=128))
            kT = [pers.tile([128, T], bf16, name=f"{r}_kT{cc}", tag=f"kT{cc}",
                            bufs=2) for cc in range(4)]
            v_ext = pers.tile([128, NJC, NH, DK], bf16, name=f"{r}_v_ext",
                              tag="v_ext", bufs=2)
            rec = pers.tile([128, 512], bf16, name=f"{r}_rec", tag="rec", bufs=2)

            qT = {}   # qT[(ib, cc)] live for one ib (ring bufs=2)

            def stage_a(ib):
                # xT via DMA xbar transpose: x[ib block, dc cols] -> [128, 512]
                if ib == 0:
                    xT = xT0
                else:
                    xT = [sb.tile([128, 512], bf16, name=f"{r}_xT_{ib}_{dc}",
                                  tag=f"xT{dc}", bufs=2) for dc in range(8)]
                    for dc in range(8):
                        nc.sync.dma_start(
                            xT[dc][:],
                            x_d[ib * 512:(ib + 1) * 512, dc * 128:(dc + 1) * 128],
                            transpose=True)
                # q/k projections -> qT (transposed, bf16, +bias), kT
                for cc in range(4):
                    pq = ps.tile([128, 512], f32, name=f"{r}_pq_{ib}_{cc}",
                                 tag="proj", bufs=1)
                    for dc in range(8):
                        nc.tensor.matmul(pq[:], wq_sb[:, dc, cc * 128:(cc + 1) * 128],
                                         xT[dc][:], start=(dc == 0), stop=(dc == 7))
                    qt = sb.tile([128, 512], bf16, name=f"{r}_qT_{ib}_{cc}",
                                 tag=f"qT{cc}", bufs=2)
                    nc.vector.tensor_scalar_add(qt[:], pq[:], bq_sb[:, cc:cc + 1])
                    qT[(ib, cc)] = qt
                    pk = ps.tile([128, 512], f32, name=f"{r}_pk_{ib}_{cc}",
                                 tag="proj", bufs=1)
                    for dc in range(8):
                        nc.tensor.matmul(pk[:], wk_sb[:, dc, cc * 128:(cc + 1) * 128],
                                         xT[dc][:], start=(dc == 0), stop=(dc == 7))
                    nc.vector.tensor_scalar_add(
                        kT[cc][:, ib * 512:(ib + 1) * 512], pk[:],
                        bk_sb[:, cc:cc + 1])
                # v projection (natural layout; no bias - folded into host const)
                for isub in range(4):
                    pv = ps.tile([128, 512], f32, name=f"{r}_pv_{ib}_{isub}",
                                 tag="proj", bufs=1)
                    for dc in range(8):
                        nc.tensor.matmul(pv[:], xT[dc][:, isub * 128:(isub + 1) * 128],
                                         wv_sb[:, dc, :], start=(dc == 0), stop=(dc == 7))
                    nc.vector.tensor_copy(
                        v_ext[:, ib * 4 + isub, :, :],
                        pv[:].rearrange("p (h d) -> p h d", d=DK))

            def stage_b(ib):
                njc = 4 * (ib + 1)
                den = ps.tile([128, 512], f32, name=f"{r}_den_{ib}", tag="den",
                              bufs=1)
                yt = []
                for hpg in range(2):
                    psys = [ps.tile([128, 512], f32, name=f"{r}_psy_{ib}_{2*hpg+i}",
                                    tag="psy", bufs=2) for i in range(2)]
                    for jc_, hp_i in [(j, i) for j in range(njc) for i in range(2)]:
                        hp = 2 * hpg + hp_i
                        psy = psys[hp_i]
                        h0, h1 = 2 * hp, 2 * hp + 1
                        jc = jc_
                        o = max(0, jc - 4 * ib)
                        i0 = o * 128
                        nw = 512 - i0
                        pss = ps.tile([128, 1024], f32, name=f"{r}_pss_{ib}_{hp}_{jc}",
                                      tag="pss", bufs=2)
                        nc.tensor.matmul(
                            pss[:, 0:nw],
                            kT[hp][0:64, jc * 128:(jc + 1) * 128],
                            qT[(ib, hp)][0:64, i0:512],
                            start=True, stop=True, tile_position=(0, 0))
                        nc.tensor.matmul(
                            pss[:, 512:512 + nw],
                            kT[hp][64:128, jc * 128:(jc + 1) * 128],
                            qT[(ib, hp)][64:128, i0:512],
                            start=True, stop=True, tile_position=(64, 0))
                        pv2 = pss[:].rearrange("p (t q) -> p t q", t=2)
                        if jc >= 4 * ib:
                            nc.vector.tensor_add(pv2[:, :, 0:128],
                                                 pv2[:, :, 0:128], invm[:])
                        et = sb.tile([128, 1024], bf16, name=f"{r}_et_{ib}_{hp}_{jc}",
                                     tag="et", bufs=4)
                        et2 = et[:].rearrange("p (t q) -> p t q", t=2)
                        nc.scalar.activation(et2[:, :, 0:nw], pv2[:, :, 0:nw],
                                             AF.Exp, scale=SCALE)
                        # each den region (32 partitions) must see its own
                        # start=True: pairs 0/1 clear the four regions, pairs
                        # 2/3 accumulate (their sel32 col for the partner head
                        # is zero, so nothing real is overwritten)
                        first = (hp < 2 and jc == 0)
                        last = (hp >= 2 and jc == njc - 1)
                        nc.tensor.matmul(
                            psy[0:64, i0:512], v_ext[:, jc, h0, :], et[:, 0:nw],
                            start=(jc == 0), stop=(jc == njc - 1),
                            skip_group_check=True, tile_position=(0, 0))
                        nc.tensor.matmul(
                            psy[64:128, i0:512], v_ext[:, jc, h1, :],
                            et[:, 512:512 + nw],
                            start=(jc == 0), stop=(jc == njc - 1),
                            skip_group_check=True, tile_position=(0, 64))
                        r0 = 32 * (h0 % 4)
                        r1 = 32 * (h1 % 4)
                        nc.tensor.matmul(
                            den[r0:r0 + 32, i0:512], sel32[:, h0 // 4, :],
                            et[:, 0:nw],
                            start=first, stop=last, skip_group_check=True,
                            tile_position=(0, r0))
                        nc.tensor.matmul(
                            den[r1:r1 + 32, i0:512], sel32[:, h1 // 4, :],
                            et[:, 512:512 + nw],
                            start=first, stop=last, skip_group_check=True,
                            tile_position=(0, r1))
                    # stash the group's y0 pairs to SBUF (frees the psys)
                    for i in range(2):
                        hp2 = 2 * hpg + i
                        y = sb.tile([128, 512], bf16, name=f"{r}_yt_{ib}_{hp2}",
                                    tag=f"yt{hp2}", bufs=2)
                        nc.vector.tensor_copy(y[:], psys[i][:])
                        yt.append(y)
                # reciprocal of the whole den bank (every row finite: the
                # sel32 duplicate-ones columns fill unused rows with sums)
                with nc.allow_low_precision(reason="1/den in bf16 is plenty"):
                    nc.vector.reciprocal(rec[:], den[:])
                packed = []
                for hp in range(4):
                    pb = ps.tile([128, 512], f32, name=f"{r}_pb_{ib}_{hp}",
                                 tag="proj", bufs=1)
                    nc.tensor.matmul(pb[:], selp[:, hp, :], rec[:],
                                     start=True, stop=True)
                    pk2 = sb.tile([128, 512], bf16, name=f"{r}_pk2_{ib}_{hp}",
                                  tag=f"packed{hp}", bufs=2)
                    nc.vector.tensor_mul(pk2[:], yt[hp][:], pb[:])
                    packed.append(pk2)
                # output projection
                for isub in range(4):
                    row0 = (ib * 4 + isub) * 128
                    osb = sb.tile([128, 1024], f32, name=f"{r}_osb_{ib}_{isub}",
                                  tag="osb", bufs=2)
                    for nb in range(2):
                        pso = ps.tile([128, 512], f32, name=f"{r}_pso_{ib}_{isub}_{nb}",
                                      tag="proj", bufs=1)
                        for cc in range(4):
                            nc.tensor.matmul(
                                pso[:],
                                packed[cc][:, isub * 128:(isub + 1) * 128],
                                wo_sb[:, cc, nb * 512:(nb + 1) * 512],
                                start=(cc == 0), stop=(cc == 3))
                        if nb == 0:
                            nc.scalar.copy(osb[:, 0:512], pso[:])
                        else:
                            nc.vector.tensor_copy(osb[:, 512:1024], pso[:])
                    nc.sync.dma_start(out_d[row0:row0 + 128, :], osb[:])

            # emission order: A0 A1 B0 A2 B1 A3 B2 B3 (A runs one ib ahead
            # so projection matmuls overlap exp-bound attention phases)
            stage_a(0)
            stage_a(1)
            stage_b(0)
            stage_a(2)
            stage_b(1)
            stage_a(3)
            stage_b(2)
            stage_b(3)

    nc.compile()
    return nc


def make_in_maps(x, wq, bq, wk, bk, wv, bv, wo, bo):
    import ml_dtypes
    bf16 = ml_dtypes.bfloat16

    jj = np.arange(128)[:, None]
    ii = np.arange(128)[None, :]
    inv_mask = np.where(jj > ii, -1e9, 0.0).astype(np.float32)
    invm2 = np.concatenate([inv_mask, inv_mask], axis=1)  # [128, 256]

    selp = np.zeros((4, 128, 128), dtype=np.float32)
    for hp in range(4):
        h0, h1 = 2 * hp, 2 * hp + 1
        selp[hp, _den_row(h0), 0:64] = 1.0
        selp[hp, _den_row(h1), 64:128] = 1.0
    # sel32[v]: column j routes et-colsums into den partition row 32g+j.
    # col v is head v's real denominator slot; col (1-v) is the other head's
    # (zeroed); cols 2..31 get harmless sums so 1/den stays finite bank-wide.
    sel32 = np.ones((2, 128, 32), dtype=np.float32)
    sel32[0, :, 1] = 0.0
    sel32[1, :, 0] = 0.0

    x = np.asarray(x, np.float32)
    in_maps = []
    for c in range(NCORES):
        b, g = c // G, c % G
        cs = slice(g * C, (g + 1) * C)
        in_maps.append({
            "x": np.ascontiguousarray(x[b]).astype(bf16),
            "wq": np.ascontiguousarray(wq[:, cs]).astype(bf16),
            "wk": np.ascontiguousarray(wk[:, cs]).astype(bf16),
            "wv": np.ascontiguousarray(wv[:, cs]).astype(bf16),
            "wo": np.ascontiguousarray(wo[cs, :]).astype(bf16),
            "bq": np.ascontiguousarray(bq[cs].reshape(C, 1)).astype(np.float32),
            "bk": np.ascontiguousarray(bk[cs].reshape(C, 1)).astype(np.float32),
            "invmask2": invm2,
            "selp": selp.astype(bf16),
            "sel32": sel32.astype(bf16),
        })
    return in_maps


_NC_CACHE = {}


def _get_nc(mm_mode=MM_MODE):
    if mm_mode not in _NC_CACHE:
        _NC_CACHE[mm_mode] = build_nc(mm_mode)
    return _NC_CACHE[mm_mode]


def kernel(x, mask, wq, bq, wk, bk, wv, bv, wo, bo, _trace=False, _results=None):
    from concourse.bass_utils import run_bass_kernel_spmd

    nc = _get_nc()
    in_maps = make_in_maps(np.asarray(x), np.asarray(wq), np.asarray(bq),
                           np.asarray(wk), np.asarray(bk), np.asarray(wv),
                           np.asarray(bv), np.asarray(wo), np.asarray(bo))
    res = run_bass_kernel_spmd(nc, in_maps, core_ids=list(range(NCORES)),
                               trace=_trace)
    if _results is not None:
        _results.append(res)
    # constant row: y += bv (since attn rows sum to 1)  =>  out += bv@wo + bo
    row_const = (np.asarray(bv, np.float64) @ np.asarray(wo, np.float64)
                 + np.asarray(bo, np.float64)).astype(np.float32)
    out = np.empty((B, T, D), dtype=np.float32)
    for b in range(B):
        out[b] = (res.results[2 * b]["out"] + res.results[2 * b + 1]["out"]
                  + row_const)
    return out


# revision 12
# speedup vs baseline: 2.0596x; 1.0772x over previous
# Multi-head causal attention (B=4, T=2048, D=1024, H=16, dk=64), fp32 in/out.
#
# Sharding: 8 cores = 4 batches x 2 head-groups (8 heads / 512 cols each).
# Each core computes a partial output  y_g @ wo_g  for its batch; the host
# sums the two head-group partials per batch and adds the constant row
# (bv @ wo + bo), which is exact because softmax rows sum to 1.
#
# v2 design (all-bf16 datapath, fp32 accumulation in PSUM):
#  - x is pre-converted to bf16 on the host and DMA-transposed (xbar) into
#    SBUF as xT [d, i] chunks - no PE transposes, no DVE copy-back.
#  - scores: row-packed head pairs (K=64 at PE rows 0/64) -> one PSUM pair
#    tile [128, 2x512]; mask add batched; one Exp activation per pair-chunk.
#  - av: column-packed head pairs (M=64 at PE cols 0/64) -> psy [128, 512]
#    holds both heads, so no cross-partition shuffle of odd heads.
#  - softmax denominators: M=32 sel-matmuls accumulated into a single PSUM
#    bank (col-tiled positions 0/32/64/96 x 2 rows), no DVE row gathering.
#  - per-ib interleaved emission (A1 ahead of B0 etc) so projection matmuls
#    fill TensorE while attention is ScalarE(exp)-bound.

import numpy as np

B, T, D, H, DK = 4, 2048, 1024, 16, 64
NCORES = 8
G = 2               # head groups (tensor-parallel over heads)
C = D // G          # 512 columns per core = 8 heads
NH = C // DK        # heads per core = 8
NIB = T // 512      # 4 query blocks of 512
NJC = T // 128      # 16 key chunks of 128
SCALE = 1.0 / 8.0   # 1/sqrt(dk)

MM_MODE = "bf16"    # kept for test.py compatibility


def _den_row(h):
    # head h's denominator lives at PSUM partition 32*(h%4) + h//4
    return 32 * (h % 4) + h // 4


def build_nc(mm_mode=MM_MODE, n_reps=1):
    from contextlib import ExitStack

    import concourse.bass as bass
    import concourse.mybir as mybir
    import concourse.tile as tile
    from concourse import bacc

    f32 = mybir.dt.float32
    bf16 = mybir.dt.bfloat16
    AF = mybir.ActivationFunctionType

    nc = bacc.Bacc("TRN2", target_bir_lowering=False, debug=False,
                   num_devices=NCORES)

    x_d = nc.dram_tensor("x", [T, D], bf16, kind="ExternalInput").ap()
    wq_d = nc.dram_tensor("wq", [D, C], bf16, kind="ExternalInput").ap()
    wk_d = nc.dram_tensor("wk", [D, C], bf16, kind="ExternalInput").ap()
    wv_d = nc.dram_tensor("wv", [D, C], bf16, kind="ExternalInput").ap()
    wo_d = nc.dram_tensor("wo", [C, D], bf16, kind="ExternalInput").ap()
    bq_d = nc.dram_tensor("bq", [C, 1], f32, kind="ExternalInput").ap()
    bk_d = nc.dram_tensor("bk", [C, 1], f32, kind="ExternalInput").ap()
    msk_d = nc.dram_tensor("invmask2", [128, 256], f32, kind="ExternalInput").ap()
    selp_d = nc.dram_tensor("selp", [4, 128, 128], bf16, kind="ExternalInput").ap()
    ones2_d = nc.dram_tensor("sel32", [2, 128, 32], bf16, kind="ExternalInput").ap()
    out_d = nc.dram_tensor("out", [T, D], f32, kind="ExternalOutput").ap()

    with tile.TileContext(nc) as tc, ExitStack() as st:
        pers = st.enter_context(tc.tile_pool(name="pers", bufs=1))
        sb = st.enter_context(tc.tile_pool(name="sb", bufs=1))
        ps = st.enter_context(tc.tile_pool(name="ps", bufs=1, space="PSUM"))

        # persistent constants (each loaded with a single batched DMA)
        bq_sb = pers.tile([128, 4], f32, name="bq_sb", tag="bq_sb")
        bk_sb = pers.tile([128, 4], f32, name="bk_sb", tag="bk_sb")
        invm = pers.tile([128, 2, 128], f32, name="invm", tag="invm")
        selp = pers.tile([128, 4, 128], bf16, name="selp", tag="selp")
        sel32 = pers.tile([128, 2, 32], bf16, name="sel32", tag="sel32")

        nc.sync.dma_start(invm[:], msk_d.rearrange("p (t q) -> p t q", t=2))
        nc.sync.dma_start(selp[:], selp_d.rearrange("hp p m -> p hp m"))
        nc.sync.dma_start(sel32[:], ones2_d.rearrange("i p m -> p i m"))
        nc.sync.dma_start(bq_sb[:], bq_d.rearrange("(cc p) one -> p (cc one)", p=128))
        nc.sync.dma_start(bk_sb[:], bk_d.rearrange("(cc p) one -> p (cc one)", p=128))

        for rep_ in range(n_reps):
            r = f"r{rep_}"
            # first block of x transposes goes out before anything else so
            # the PE can start projecting ASAP
            xT0 = [sb.tile([128, 512], bf16, name=f"{r}_xT_0_{dc}",
                           tag=f"xT{dc}", bufs=2) for dc in range(8)]
            for dc in range(8):
                nc.sync.dma_start(
                    xT0[dc][:], x_d[0:512, dc * 128:(dc + 1) * 128],
                    transpose=True)
            # weights: one batched DMA per tensor, double-buffered so the
            # next rep's loads overlap this rep's tail
            wq_sb = sb.tile([128, 8, C], bf16, name=f"{r}_wq", tag="wq", bufs=2)
            wk_sb = sb.tile([128, 8, C], bf16, name=f"{r}_wk", tag="wk", bufs=2)
            wv_sb = sb.tile([128, 8, C], bf16, name=f"{r}_wv", tag="wv", bufs=2)
            wo_sb = sb.tile([128, 4, D], bf16, name=f"{r}_wo", tag="wo", bufs=2)
            nc.sync.dma_start(wq_sb[:], wq_d.rearrange("(dc p) n -> p dc n", p=128))
            nc.sync.dma_start(wk_sb[:], wk_d.rearrange("(dc p) n -> p dc n", p=128))
            nc.sync.dma_start(wv_sb[:], wv_d.rearrange("(dc p) n -> p dc n", p=128))
            nc.sync.dma_start(wo_sb[:], wo_d.rearrange("(cc p) n -> p cc n", p=128))
            kT = [pers.tile([128, T], bf16, name=f"{r}_kT{cc}", tag=f"kT{cc}",
                            bufs=2) for cc in range(4)]
            v_ext = pers.tile([128, NJC, NH, DK], bf16, name=f"{r}_v_ext",
                              tag="v_ext", bufs=2)
            rec = pers.tile([128, 512], bf16, name=f"{r}_rec", tag="rec", bufs=2)

            qT = {}   # qT[(ib, cc)] live for one ib (ring bufs=2)

            def stage_a(ib):
                # xT via DMA xbar transpose: x[ib block, dc cols] -> [128, 512]
                if ib == 0:
                    xT = xT0
                else:
                    xT = [sb.tile([128, 512], bf16, name=f"{r}_xT_{ib}_{dc}",
                                  tag=f"xT{dc}", bufs=2) for dc in range(8)]
                    for dc in range(8):
                        nc.sync.dma_start(
                            xT[dc][:],
                            x_d[ib * 512:(ib + 1) * 512, dc * 128:(dc + 1) * 128],
                            transpose=True)
                # q/k projections -> qT (transposed, bf16, +bias), kT
                for cc in range(4):
                    pq = ps.tile([128, 512], f32, name=f"{r}_pq_{ib}_{cc}",
                                 tag="proj", bufs=1)
                    for dc in range(8):
                        nc.tensor.matmul(pq[:], wq_sb[:, dc, cc * 128:(cc + 1) * 128],
                                         xT[dc][:], start=(dc == 0), stop=(dc == 7))
                    qt = sb.tile([128, 512], bf16, name=f"{r}_qT_{ib}_{cc}",
                                 tag=f"qT{cc}", bufs=2)
                    nc.vector.tensor_scalar_add(qt[:], pq[:], bq_sb[:, cc:cc + 1])
                    qT[(ib, cc)] = qt
                    pk = ps.tile([128, 512], f32, name=f"{r}_pk_{ib}_{cc}",
                                 tag="proj", bufs=1)
                    for dc in range(8):
                        nc.tensor.matmul(pk[:], wk_sb[:, dc, cc * 128:(cc + 1) * 128],
                                         xT[dc][:], start=(dc == 0), stop=(dc == 7))
                    nc.vector.tensor_scalar_add(
                        kT[cc][:, ib * 512:(ib + 1) * 512], pk[:],
                        bk_sb[:, cc:cc + 1])
                # v projection (natural layout; no bias - folded into host const)
                for isub in range(4):
                    pv = ps.tile([128, 512], f32, name=f"{r}_pv_{ib}_{isub}",
                                 tag="proj", bufs=1)
                    for dc in range(8):
                        nc.tensor.matmul(pv[:], xT[dc][:, isub * 128:(isub + 1) * 128],
                                         wv_sb[:, dc, :], start=(dc == 0), stop=(dc == 7))
                    nc.vector.tensor_copy(
                        v_ext[:, ib * 4 + isub, :, :],
                        pv[:].rearrange("p (h d) -> p h d", d=DK))

            def stage_b(ib):
                njc = 4 * (ib + 1)
                den = ps.tile([128, 512], f32, name=f"{r}_den_{ib}", tag="den",
                              bufs=1)
                yt = []
                for hpg in range(2):
                    psys = [ps.tile([128, 512], f32, name=f"{r}_psy_{ib}_{2*hpg+i}",
                                    tag="psy", bufs=2) for i in range(2)]
                    for jc_, hp_i in [(j, i) for j in range(njc) for i in range(2)]:
                        hp = 2 * hpg + hp_i
                        psy = psys[hp_i]
                        h0, h1 = 2 * hp, 2 * hp + 1
                        jc = jc_
                        o = max(0, jc - 4 * ib)
                        i0 = o * 128
                        nw = 512 - i0
                        pss = ps.tile([128, 1024], f32, name=f"{r}_pss_{ib}_{hp}_{jc}",
                                      tag="pss", bufs=2)
                        nc.tensor.matmul(
                            pss[:, 0:nw],
                            kT[hp][0:64, jc * 128:(jc + 1) * 128],
                            qT[(ib, hp)][0:64, i0:512],
                            start=True, stop=True, tile_position=(0, 0))
                        nc.tensor.matmul(
                            pss[:, 512:512 + nw],
                            kT[hp][64:128, jc * 128:(jc + 1) * 128],
                            qT[(ib, hp)][64:128, i0:512],
                            start=True, stop=True, tile_position=(64, 0))
                        pv2 = pss[:].rearrange("p (t q) -> p t q", t=2)
                        if jc >= 4 * ib:
                            nc.vector.tensor_add(pv2[:, :, 0:128],
                                                 pv2[:, :, 0:128], invm[:])
                        et = sb.tile([128, 1024], bf16, name=f"{r}_et_{ib}_{hp}_{jc}",
                                     tag="et", bufs=6)
                        et2 = et[:].rearrange("p (t q) -> p t q", t=2)
                        nc.scalar.activation(et2[:, :, 0:nw], pv2[:, :, 0:nw],
                                             AF.Exp, scale=SCALE)
                        # each den region (32 partitions) must see its own
                        # start=True: pairs 0/1 clear the four regions, pairs
                        # 2/3 accumulate (their sel32 col for the partner head
                        # is zero, so nothing real is overwritten)
                        first = (hp < 2 and jc == 0)
                        last = (hp >= 2 and jc == njc - 1)
                        nc.tensor.matmul(
                            psy[0:64, i0:512], v_ext[:, jc, h0, :], et[:, 0:nw],
                            start=(jc == 0), stop=(jc == njc - 1),
                            skip_group_check=True, tile_position=(0, 0))
                        nc.tensor.matmul(
                            psy[64:128, i0:512], v_ext[:, jc, h1, :],
                            et[:, 512:512 + nw],
                            start=(jc == 0), stop=(jc == njc - 1),
                            skip_group_check=True, tile_position=(0, 64))
                        r0 = 32 * (h0 % 4)
                        r1 = 32 * (h1 % 4)
                        nc.tensor.matmul(
                            den[r0:r0 + 32, i0:512], sel32[:, h0 // 4, :],
                            et[:, 0:nw],
                            start=first, stop=last, skip_group_check=True,
                            tile_position=(0, r0))
                        nc.tensor.matmul(
                            den[r1:r1 + 32, i0:512], sel32[:, h1 // 4, :],
                            et[:, 512:512 + nw],
                            start=first, stop=last, skip_group_check=True,
                            tile_position=(0, r1))
                    # stash the group's y0 pairs to SBUF (frees the psys)
                    for i in range(2):
                        hp2 = 2 * hpg + i
                        y = sb.tile([128, 512], bf16, name=f"{r}_yt_{ib}_{hp2}",
                                    tag=f"yt{hp2}", bufs=2)
                        nc.vector.tensor_copy(y[:], psys[i][:])
                        yt.append(y)
                # reciprocal of the whole den bank (every row finite: the
                # sel32 duplicate-ones columns fill unused rows with sums)
                with nc.allow_low_precision(reason="1/den in bf16 is plenty"):
                    nc.vector.reciprocal(rec[:], den[:])
                packed = []
                for hp in range(4):
                    pb = ps.tile([128, 512], f32, name=f"{r}_pb_{ib}_{hp}",
                                 tag="proj", bufs=1)
                    nc.tensor.matmul(pb[:], selp[:, hp, :], rec[:],
                                     start=True, stop=True)
                    pk2 = sb.tile([128, 512], bf16, name=f"{r}_pk2_{ib}_{hp}",
                                  tag=f"packed{hp}", bufs=2)
                    nc.vector.tensor_mul(pk2[:], yt[hp][:], pb[:])
                    packed.append(pk2)
                # output projection (osb drains on DVE - ScalarE stays free
                # for the exp stream)
                for isub in range(4):
                    row0 = (ib * 4 + isub) * 128
                    osb = sb.tile([128, 1024], f32, name=f"{r}_osb_{ib}_{isub}",
                                  tag="osb", bufs=2)
                    for nb in range(2):
                        pso = ps.tile([128, 512], f32, name=f"{r}_pso_{ib}_{isub}_{nb}",
                                      tag="proj", bufs=1)
                        for cc in range(4):
                            nc.tensor.matmul(
                                pso[:],
                                packed[cc][:, isub * 128:(isub + 1) * 128],
                                wo_sb[:, cc, nb * 512:(nb + 1) * 512],
                                start=(cc == 0), stop=(cc == 3))
                        nc.vector.tensor_copy(osb[:, nb * 512:(nb + 1) * 512], pso[:])
                    nc.sync.dma_start(out_d[row0:row0 + 128, :], osb[:])

            # emission order: A0 A1 B0 A2 B1 A3 B2 B3 (A runs one ib ahead
            # so projection matmuls overlap exp-bound attention phases)
            stage_a(0)
            stage_a(1)
            stage_b(0)
            stage_a(2)
            stage_b(1)
            stage_a(3)
            stage_b(2)
            stage_b(3)

    nc.compile()
    return nc


def make_in_maps(x, wq, bq, wk, bk, wv, bv, wo, bo):
    import ml_dtypes
    bf16 = ml_dtypes.bfloat16

    jj = np.arange(128)[:, None]
    ii = np.arange(128)[None, :]
    inv_mask = np.where(jj > ii, -1e9, 0.0).astype(np.float32)
    invm2 = np.concatenate([inv_mask, inv_mask], axis=1)  # [128, 256]

    selp = np.zeros((4, 128, 128), dtype=np.float32)
    for hp in range(4):
        h0, h1 = 2 * hp, 2 * hp + 1
        selp[hp, _den_row(h0), 0:64] = 1.0
        selp[hp, _den_row(h1), 64:128] = 1.0
    # sel32[v]: column j routes et-colsums into den partition row 32g+j.
    # col v is head v's real denominator slot; col (1-v) is the other head's
    # (zeroed); cols 2..31 get harmless sums so 1/den stays finite bank-wide.
    sel32 = np.ones((2, 128, 32), dtype=np.float32)
    sel32[0, :, 1] = 0.0
    sel32[1, :, 0] = 0.0

    x = np.asarray(x, np.float32)
    in_maps = []
    for c in range(NCORES):
        b, g = c // G, c % G
        cs = slice(g * C, (g + 1) * C)
        in_maps.append({
            "x": np.ascontiguousarray(x[b]).astype(bf16),
            "wq": np.ascontiguousarray(wq[:, cs]).astype(bf16),
            "wk": np.ascontiguousarray(wk[:, cs]).astype(bf16),
            "wv": np.ascontiguousarray(wv[:, cs]).astype(bf16),
            "wo": np.ascontiguousarray(wo[cs, :]).astype(bf16),
            "bq": np.ascontiguousarray(bq[cs].reshape(C, 1)).astype(np.float32),
            "bk": np.ascontiguousarray(bk[cs].reshape(C, 1)).astype(np.float32),
            "invmask2": invm2,
            "selp": selp.astype(bf16),
            "sel32": sel32.astype(bf16),
        })
    return in_maps


_NC_CACHE = {}


def _get_nc(mm_mode=MM_MODE):
    if mm_mode not in _NC_CACHE:
        _NC_CACHE[mm_mode] = build_nc(mm_mode)
    return _NC_CACHE[mm_mode]


def kernel(x, mask, wq, bq, wk, bk, wv, bv, wo, bo, _trace=False, _results=None):
    from concourse.bass_utils import run_bass_kernel_spmd

    nc = _get_nc()
    in_maps = make_in_maps(np.asarray(x), np.asarray(wq), np.asarray(bq),
                           np.asarray(wk), np.asarray(bk), np.asarray(wv),
                           np.asarray(bv), np.asarray(wo), np.asarray(bo))
    res = run_bass_kernel_spmd(nc, in_maps, core_ids=list(range(NCORES)),
                               trace=_trace)
    if _results is not None:
        _results.append(res)
    # constant row: y += bv (since attn rows sum to 1)  =>  out += bv@wo + bo
    row_const = (np.asarray(bv, np.float64) @ np.asarray(wo, np.float64)
                 + np.asarray(bo, np.float64)).astype(np.float32)
    out = np.empty((B, T, D), dtype=np.float32)
    for b in range(B):
        out[b] = (res.results[2 * b]["out"] + res.results[2 * b + 1]["out"]
                  + row_const)
    return out


# revision 14
# speedup vs baseline: 2.5635x; 1.2446x over previous
# Multi-head causal attention (B=4, T=2048, D=1024, H=16, dk=64), fp32 in/out.
#
# Sharding: 8 cores = 4 batches x 2 head-groups (8 heads / 512 cols each).
# Each core computes a partial output  y_g @ wo_g  for its batch; the host
# sums the two head-group partials per batch and adds the constant row
# (bv @ wo + bo), which is exact because softmax rows sum to 1.
#
# v2 design (all-bf16 datapath, fp32 accumulation in PSUM):
#  - x is pre-converted to bf16 AND pre-transposed on the host, so xT
#    [d, i] chunks load as plain contiguous DMAs - no PE transposes, no
#    DVE copy-back, no xbar.
#  - scores: row-packed head pairs (K=64 at PE rows 0/64) -> one PSUM pair
#    tile [128, 2x512]; mask add batched; one Exp activation per pair-chunk.
#  - av: column-packed head pairs (M=64 at PE cols 0/64) -> psy [128, 512]
#    holds both heads, so no cross-partition shuffle of odd heads.
#  - softmax denominators: M=32 sel-matmuls accumulated into a single PSUM
#    bank (col-tiled positions 0/32/64/96 x 2 rows), no DVE row gathering.
#  - per-ib interleaved emission (A1 ahead of B0 etc) so projection matmuls
#    fill TensorE while attention is ScalarE(exp)-bound.

import numpy as np

B, T, D, H, DK = 4, 2048, 1024, 16, 64
NCORES = 8
G = 2               # head groups (tensor-parallel over heads)
C = D // G          # 512 columns per core = 8 heads
NH = C // DK        # heads per core = 8
NIB = T // 512      # 4 query blocks of 512
NJC = T // 128      # 16 key chunks of 128
SCALE = 1.0 / 8.0   # 1/sqrt(dk)

MM_MODE = "bf16"    # kept for test.py compatibility


def _den_row(h):
    # head h's denominator lives at PSUM partition 32*(h%4) + h//4
    return 32 * (h % 4) + h // 4


def build_nc(mm_mode=MM_MODE, n_reps=1):
    from contextlib import ExitStack

    import concourse.bass as bass
    import concourse.mybir as mybir
    import concourse.tile as tile
    from concourse import bacc

    f32 = mybir.dt.float32
    bf16 = mybir.dt.bfloat16
    AF = mybir.ActivationFunctionType

    nc = bacc.Bacc("TRN2", target_bir_lowering=False, debug=False,
                   num_devices=NCORES)

    x_d = nc.dram_tensor("x", [D, T], bf16, kind="ExternalInput").ap()  # pre-transposed on host
    wq_d = nc.dram_tensor("wq", [D, C], bf16, kind="ExternalInput").ap()
    wk_d = nc.dram_tensor("wk", [D, C], bf16, kind="ExternalInput").ap()
    wv_d = nc.dram_tensor("wv", [D, C], bf16, kind="ExternalInput").ap()
    wo_d = nc.dram_tensor("wo", [C, D], bf16, kind="ExternalInput").ap()
    bq_d = nc.dram_tensor("bq", [C, 1], f32, kind="ExternalInput").ap()
    bk_d = nc.dram_tensor("bk", [C, 1], f32, kind="ExternalInput").ap()
    msk_d = nc.dram_tensor("invmask2", [128, 256], f32, kind="ExternalInput").ap()
    selp_d = nc.dram_tensor("selp", [4, 128, 128], bf16, kind="ExternalInput").ap()
    ones2_d = nc.dram_tensor("sel32", [2, 128, 32], bf16, kind="ExternalInput").ap()
    out_d = nc.dram_tensor("out", [T, D], f32, kind="ExternalOutput").ap()

    with tile.TileContext(nc) as tc, ExitStack() as st:
        pers = st.enter_context(tc.tile_pool(name="pers", bufs=1))
        sb = st.enter_context(tc.tile_pool(name="sb", bufs=1))
        ps = st.enter_context(tc.tile_pool(name="ps", bufs=1, space="PSUM"))

        # persistent constants (each loaded with a single batched DMA)
        bq_sb = pers.tile([128, 4], f32, name="bq_sb", tag="bq_sb")
        bk_sb = pers.tile([128, 4], f32, name="bk_sb", tag="bk_sb")
        invm = pers.tile([128, 2, 128], f32, name="invm", tag="invm")
        selp = pers.tile([128, 4, 128], bf16, name="selp", tag="selp")
        sel32 = pers.tile([128, 2, 32], bf16, name="sel32", tag="sel32")

        nc.sync.dma_start(invm[:], msk_d.rearrange("p (t q) -> p t q", t=2))
        nc.sync.dma_start(selp[:], selp_d.rearrange("hp p m -> p hp m"))
        nc.sync.dma_start(sel32[:], ones2_d.rearrange("i p m -> p i m"))
        nc.sync.dma_start(bq_sb[:], bq_d.rearrange("(cc p) one -> p (cc one)", p=128))
        nc.sync.dma_start(bk_sb[:], bk_d.rearrange("(cc p) one -> p (cc one)", p=128))

        for rep_ in range(n_reps):
            r = f"r{rep_}"
            # first block of x transposes goes out before anything else so
            # the PE can start projecting ASAP
            xT0 = [sb.tile([128, 512], bf16, name=f"{r}_xT_0_{dc}",
                           tag=f"xT{dc}", bufs=2) for dc in range(8)]
            for dc in range(8):
                nc.sync.dma_start(
                    xT0[dc][:], x_d[dc * 128:(dc + 1) * 128, 0:512])
            # weights: one batched DMA per tensor, double-buffered so the
            # next rep's loads overlap this rep's tail
            wq_sb = sb.tile([128, 8, C], bf16, name=f"{r}_wq", tag="wq", bufs=2)
            wk_sb = sb.tile([128, 8, C], bf16, name=f"{r}_wk", tag="wk", bufs=2)
            wv_sb = sb.tile([128, 8, C], bf16, name=f"{r}_wv", tag="wv", bufs=2)
            wo_sb = sb.tile([128, 4, D], bf16, name=f"{r}_wo", tag="wo", bufs=2)
            nc.sync.dma_start(wq_sb[:], wq_d.rearrange("(dc p) n -> p dc n", p=128))
            nc.sync.dma_start(wk_sb[:], wk_d.rearrange("(dc p) n -> p dc n", p=128))
            nc.sync.dma_start(wv_sb[:], wv_d.rearrange("(dc p) n -> p dc n", p=128))
            nc.sync.dma_start(wo_sb[:], wo_d.rearrange("(cc p) n -> p cc n", p=128))
            kT = [pers.tile([128, T], bf16, name=f"{r}_kT{cc}", tag=f"kT{cc}",
                            bufs=2) for cc in range(4)]
            v_ext = pers.tile([128, NJC, NH, DK], bf16, name=f"{r}_v_ext",
                              tag="v_ext", bufs=2)
            rec = pers.tile([128, 512], bf16, name=f"{r}_rec", tag="rec", bufs=2)

            qT = {}   # qT[(ib, cc)] live for one ib (ring bufs=2)

            def stage_a(ib):
                # xT chunks: plain contiguous loads from host-pre-transposed x
                if ib == 0:
                    xT = xT0
                else:
                    xT = [sb.tile([128, 512], bf16, name=f"{r}_xT_{ib}_{dc}",
                                  tag=f"xT{dc}", bufs=2) for dc in range(8)]
                    for dc in range(8):
                        nc.sync.dma_start(
                            xT[dc][:],
                            x_d[dc * 128:(dc + 1) * 128,
                                ib * 512:(ib + 1) * 512])
                # q/k projections -> qT (transposed, bf16, +bias), kT
                for cc in range(4):
                    pq = ps.tile([128, 512], f32, name=f"{r}_pq_{ib}_{cc}",
                                 tag="proj", bufs=1)
                    for dc in range(8):
                        nc.tensor.matmul(pq[:], wq_sb[:, dc, cc * 128:(cc + 1) * 128],
                                         xT[dc][:], start=(dc == 0), stop=(dc == 7))
                    qt = sb.tile([128, 512], bf16, name=f"{r}_qT_{ib}_{cc}",
                                 tag=f"qT{cc}", bufs=2)
                    nc.vector.tensor_scalar_add(qt[:], pq[:], bq_sb[:, cc:cc + 1])
                    qT[(ib, cc)] = qt
                    pk = ps.tile([128, 512], f32, name=f"{r}_pk_{ib}_{cc}",
                                 tag="proj", bufs=1)
                    for dc in range(8):
                        nc.tensor.matmul(pk[:], wk_sb[:, dc, cc * 128:(cc + 1) * 128],
                                         xT[dc][:], start=(dc == 0), stop=(dc == 7))
                    nc.vector.tensor_scalar_add(
                        kT[cc][:, ib * 512:(ib + 1) * 512], pk[:],
                        bk_sb[:, cc:cc + 1])
                # v projection (natural layout; no bias - folded into host const)
                for isub in range(4):
                    pv = ps.tile([128, 512], f32, name=f"{r}_pv_{ib}_{isub}",
                                 tag="proj", bufs=1)
                    for dc in range(8):
                        nc.tensor.matmul(pv[:], xT[dc][:, isub * 128:(isub + 1) * 128],
                                         wv_sb[:, dc, :], start=(dc == 0), stop=(dc == 7))
                    nc.vector.tensor_copy(
                        v_ext[:, ib * 4 + isub, :, :],
                        pv[:].rearrange("p (h d) -> p h d", d=DK))

            def stage_b(ib):
                njc = 4 * (ib + 1)
                den = ps.tile([128, 512], f32, name=f"{r}_den_{ib}", tag="den",
                              bufs=1)
                yt = []
                for hpg in range(2):
                    psys = [ps.tile([128, 512], f32, name=f"{r}_psy_{ib}_{2*hpg+i}",
                                    tag="psy", bufs=2) for i in range(2)]
                    for jc_, hp_i in [(j, i) for j in range(njc) for i in range(2)]:
                        hp = 2 * hpg + hp_i
                        psy = psys[hp_i]
                        h0, h1 = 2 * hp, 2 * hp + 1
                        jc = jc_
                        o = max(0, jc - 4 * ib)
                        i0 = o * 128
                        nw = 512 - i0
                        pss = ps.tile([128, 1024], f32, name=f"{r}_pss_{ib}_{hp}_{jc}",
                                      tag="pss", bufs=2)
                        nc.tensor.matmul(
                            pss[:, 0:nw],
                            kT[hp][0:64, jc * 128:(jc + 1) * 128],
                            qT[(ib, hp)][0:64, i0:512],
                            start=True, stop=True, tile_position=(0, 0))
                        nc.tensor.matmul(
                            pss[:, 512:512 + nw],
                            kT[hp][64:128, jc * 128:(jc + 1) * 128],
                            qT[(ib, hp)][64:128, i0:512],
                            start=True, stop=True, tile_position=(64, 0))
                        pv2 = pss[:].rearrange("p (t q) -> p t q", t=2)
                        if jc >= 4 * ib:
                            nc.vector.tensor_add(pv2[:, :, 0:128],
                                                 pv2[:, :, 0:128], invm[:])
                        et = sb.tile([128, 1024], bf16, name=f"{r}_et_{ib}_{hp}_{jc}",
                                     tag="et", bufs=6)
                        et2 = et[:].rearrange("p (t q) -> p t q", t=2)
                        nc.scalar.activation(et2[:, :, 0:nw], pv2[:, :, 0:nw],
                                             AF.Exp, scale=SCALE)
                        # each den region (32 partitions) must see its own
                        # start=True: pairs 0/1 clear the four regions, pairs
                        # 2/3 accumulate (their sel32 col for the partner head
                        # is zero, so nothing real is overwritten)
                        first = (hp < 2 and jc == 0)
                        last = (hp >= 2 and jc == njc - 1)
                        nc.tensor.matmul(
                            psy[0:64, i0:512], v_ext[:, jc, h0, :], et[:, 0:nw],
                            start=(jc == 0), stop=(jc == njc - 1),
                            skip_group_check=True, tile_position=(0, 0))
                        nc.tensor.matmul(
                            psy[64:128, i0:512], v_ext[:, jc, h1, :],
                            et[:, 512:512 + nw],
                            start=(jc == 0), stop=(jc == njc - 1),
                            skip_group_check=True, tile_position=(0, 64))
                        r0 = 32 * (h0 % 4)
                        r1 = 32 * (h1 % 4)
                        nc.tensor.matmul(
                            den[r0:r0 + 32, i0:512], sel32[:, h0 // 4, :],
                            et[:, 0:nw],
                            start=first, stop=last, skip_group_check=True,
                            tile_position=(0, r0))
                        nc.tensor.matmul(
                            den[r1:r1 + 32, i0:512], sel32[:, h1 // 4, :],
                            et[:, 512:512 + nw],
                            start=first, stop=last, skip_group_check=True,
                            tile_position=(0, r1))
                    # stash the group's y0 pairs to SBUF (frees the psys)
                    for i in range(2):
                        hp2 = 2 * hpg + i
                        y = sb.tile([128, 512], bf16, name=f"{r}_yt_{ib}_{hp2}",
                                    tag=f"yt{hp2}", bufs=2)
                        nc.vector.tensor_copy(y[:], psys[i][:])
                        yt.append(y)
                # reciprocal of the whole den bank (every row finite: the
                # sel32 duplicate-ones columns fill unused rows with sums)
                with nc.allow_low_precision(reason="1/den in bf16 is plenty"):
                    nc.vector.reciprocal(rec[:], den[:])
                packed = []
                for hp in range(4):
                    pb = ps.tile([128, 512], f32, name=f"{r}_pb_{ib}_{hp}",
                                 tag="proj", bufs=1)
                    nc.tensor.matmul(pb[:], selp[:, hp, :], rec[:],
                                     start=True, stop=True)
                    pk2 = sb.tile([128, 512], bf16, name=f"{r}_pk2_{ib}_{hp}",
                                  tag=f"packed{hp}", bufs=2)
                    nc.vector.tensor_mul(pk2[:], yt[hp][:], pb[:])
                    packed.append(pk2)
                # output projection (osb drains on DVE - ScalarE stays free
                # for the exp stream)
                for isub in range(4):
                    row0 = (ib * 4 + isub) * 128
                    osb = sb.tile([128, 1024], f32, name=f"{r}_osb_{ib}_{isub}",
                                  tag="osb", bufs=2)
                    for nb in range(2):
                        pso = ps.tile([128, 512], f32, name=f"{r}_pso_{ib}_{isub}_{nb}",
                                      tag="proj", bufs=1)
                        for cc in range(4):
                            nc.tensor.matmul(
                                pso[:],
                                packed[cc][:, isub * 128:(isub + 1) * 128],
                                wo_sb[:, cc, nb * 512:(nb + 1) * 512],
                                start=(cc == 0), stop=(cc == 3))
                        nc.vector.tensor_copy(osb[:, nb * 512:(nb + 1) * 512], pso[:])
                    nc.sync.dma_start(out_d[row0:row0 + 128, :], osb[:])

            # emission order: A0 A1 B0 A2 B1 A3 B2 B3 (A runs one ib ahead
            # so projection matmuls overlap exp-bound attention phases)
            stage_a(0)
            stage_a(1)
            stage_b(0)
            stage_a(2)
            stage_b(1)
            stage_a(3)
            stage_b(2)
            stage_b(3)

    nc.compile()
    return nc


def make_in_maps(x, wq, bq, wk, bk, wv, bv, wo, bo):
    import ml_dtypes
    bf16 = ml_dtypes.bfloat16

    jj = np.arange(128)[:, None]
    ii = np.arange(128)[None, :]
    inv_mask = np.where(jj > ii, -1e9, 0.0).astype(np.float32)
    invm2 = np.concatenate([inv_mask, inv_mask], axis=1)  # [128, 256]

    selp = np.zeros((4, 128, 128), dtype=np.float32)
    for hp in range(4):
        h0, h1 = 2 * hp, 2 * hp + 1
        selp[hp, _den_row(h0), 0:64] = 1.0
        selp[hp, _den_row(h1), 64:128] = 1.0
    # sel32[v]: column j routes et-colsums into den partition row 32g+j.
    # col v is head v's real denominator slot; col (1-v) is the other head's
    # (zeroed); cols 2..31 get harmless sums so 1/den stays finite bank-wide.
    sel32 = np.ones((2, 128, 32), dtype=np.float32)
    sel32[0, :, 1] = 0.0
    sel32[1, :, 0] = 0.0

    x = np.asarray(x, np.float32)
    in_maps = []
    for c in range(NCORES):
        b, g = c // G, c % G
        cs = slice(g * C, (g + 1) * C)
        in_maps.append({
            "x": np.ascontiguousarray(x[b].T.astype(bf16)),
            "wq": np.ascontiguousarray(wq[:, cs]).astype(bf16),
            "wk": np.ascontiguousarray(wk[:, cs]).astype(bf16),
            "wv": np.ascontiguousarray(wv[:, cs]).astype(bf16),
            "wo": np.ascontiguousarray(wo[cs, :]).astype(bf16),
            "bq": np.ascontiguousarray(bq[cs].reshape(C, 1)).astype(np.float32),
            "bk": np.ascontiguousarray(bk[cs].reshape(C, 1)).astype(np.float32),
            "invmask2": invm2,
            "selp": selp.astype(bf16),
            "sel32": sel32.astype(bf16),
        })
    return in_maps


_NC_CACHE = {}


def _get_nc(mm_mode=MM_MODE):
    if mm_mode not in _NC_CACHE:
        _NC_CACHE[mm_mode] = build_nc(mm_mode)
    return _NC_CACHE[mm_mode]


def kernel(x, mask, wq, bq, wk, bk, wv, bv, wo, bo, _trace=False, _results=None):
    from concourse.bass_utils import run_bass_kernel_spmd

    nc = _get_nc()
    in_maps = make_in_maps(np.asarray(x), np.asarray(wq), np.asarray(bq),
                           np.asarray(wk), np.asarray(bk), np.asarray(wv),
                           np.asarray(bv), np.asarray(wo), np.asarray(bo))
    res = run_bass_kernel_spmd(nc, in_maps, core_ids=list(range(NCORES)),
                               trace=_trace)
    if _results is not None:
        _results.append(res)
    # constant row: y += bv (since attn rows sum to 1)  =>  out += bv@wo + bo
    row_const = (np.asarray(bv, np.float64) @ np.asarray(wo, np.float64)
                 + np.asarray(bo, np.float64)).astype(np.float32)
    out = np.empty((B, T, D), dtype=np.float32)
    for b in range(B):
        out[b] = (res.results[2 * b]["out"] + res.results[2 * b + 1]["out"]
                  + row_const)
    return out
